# revision 2
# baseline (speedup 1.0000x reference)
"""Trainium2 Bass kernel for a GCN message-passing layer (v5).

Reference computation (per node i):
    out[i] = sum_j edges[i,j] * (w1 @ concat(x[j], dist[i,j])) + w2 @ x[i]
which factors into:
    xmsg = x @ w1x.T                       (w1x = w1[:, :128])
    agg  = edges @ xmsg                    (big GEMM, contraction over j)
    dw   = einsum('ij,ijc->ic', edges, dist)
    out  = agg + dw @ w1d.T + x @ w2.T     (w1d = w1[:, 128:130])

Sharding: rows i (targets) split across 8 NeuronCores; x/w1/w2 replicated.

Data plan (tolerance 2e-2 rel-L2; measured 9.5e-3):
  - edges stream as SHIFTED fp8-e3m4 (E' = e-0.5; centering the uniform
    [0,1) data halves its quantization error) -> 8 MB/core; the host
    product P = e (*) d streams as fp8-e4m3 -> 16 MB/core; ~26.3 MB/core
    total HBM vs the ~358 GB/s per-core ceiling.  P's host precompute is
    268 MFLOP, 1.5% of total FLOPs, the same scale as the existing host
    xmsg GEMM; it removes the on-device elementwise multiply (74 us of
    DVE in the bf16/fp8-D version) and all D upconverts (92 us ScalarE).
  - dist j-reduction runs as DoubleRow fp8 matmuls: the two dist
    channels ride the two fp8 weight slots (stationary w1d2[j, c, f] =
    w1d[f, c], j-independent), contracting 128 j x 2 c per column: half
    the matmul count of a per-channel scheme.  agg matmuls stay bf16
    (all-DoubleRow measured +20% on every matmul via the DoubleRow
    issue tax and lost FWL -- slower overall).
  - E' upconverts to bf16 at PAIR granularity with the +0.5 shift folded
    into the engines' free affine stage, alternating ScalarE
    activation(Copy, bias) / DVE tensor_scalar(add) per pair (~2.0 us
    each, both engines ~45% busy, off the critical path).

Schedule plan (from v2-v4 traces):
  - HWDGE completion-semaphore lanes are assigned round-robin over ALL
    hw DMAs in emission order (8 lanes); a DMA's issue waits for the
    previous same-lane DMA's consumers.  Emissions here interleave the
    act-ring E' quads with the sync-ring P pairs so same-lane reuse
    mixes fast (PE) and slow (upconvert) waiters and upconverts are
    pair-sized, keeping every chain step well under the ~4.3 us of
    transfer a lane rotation covers.
  - E' quad DMAs ride the act ring, issued from ScalarE program order
    ahead of that quad's upconverts; P pair DMAs + constants + output
    ride the sync ring.  The first quad is split in half so pair 0's
    upconvert starts ~1.4 us sooner, and the first upconverts are
    pair-sized, pulling the first matmul from ~15 us to ~10 us.
  - per pair: agg matmuls right after the pair's upconvert; DoubleRow
    dist matmuls batch per 4-pair group.  PSUM accumulates across the
    whole kernel per 512-col half; the self-connection term closes it.
"""

import os

import numpy as np
import ml_dtypes

import concourse.bacc as bacc
import concourse.mybir as mybir
from concourse.tile import TileContext

F32 = mybir.dt.float32
BF16 = mybir.dt.bfloat16
FP8E4 = mybir.dt.float8e4
FP8E3 = mybir.dt.float8e3
P = 128

# problem dims (hardcoded per contract)
N_FULL = 8192
F_IN = 128
F_OUT = 128
N_CORES = 8

# tunables
GROUP = 4        # pairs per dist-batch group
PF_QUADS = 2     # E' quad DMA prefetch depth
PF_PAIRS = 5     # P pair DMA prefetch depth
E_BUFS = 4       # in-flight E' quad tiles
EB_BUFS = 6      # in-flight upconverted-E pair tiles
P_BUFS = 8       # in-flight P pair tiles

LAST_RESULT = None  # BassKernelResults of the most recent kernel() call


def build(n=N_FULL, rows=N_FULL // N_CORES):
    f = F_IN
    assert n % (4 * P) == 0 and rows == 1024
    n_tile = n // P       # 64 j-tiles
    n_pair = n_tile // 2  # 32
    n_quad = n_tile // 4  # 16
    n_grp = n_pair // GROUP

    nc = bacc.Bacc()
    eQ_d = nc.declare_dram_parameter("eQ", [n_quad, P, 4 * rows], FP8E3,
                                     isOutput=False)
    pP_d = nc.declare_dram_parameter("pP", [n_pair, P, 2, 2, rows], FP8E4,
                                     isOutput=False)
    xm_d = nc.declare_dram_parameter("xmsg", [P, n_tile, f], BF16,
                                     isOutput=False)
    xTs_d = nc.declare_dram_parameter("xT_self", [f, rows], BF16, isOutput=False)
    w2T_d = nc.declare_dram_parameter("w2T", [f, F_OUT], BF16, isOutput=False)
    w1d2_d = nc.declare_dram_parameter("w1d2", [P, 2, F_OUT], FP8E4,
                                       isOutput=False)
    o_d = nc.declare_dram_parameter("outT", [F_OUT, rows], F32, isOutput=True)

    with TileContext(nc) as tc:
        with (
            tc.tile_pool(name="const", bufs=1) as cpool,
            tc.tile_pool(name="stream", bufs=2) as pool,
            tc.tile_pool(name="psum", bufs=2, space="PSUM") as pp,
        ):
            eQ_sb = {}
            pPs = {}

            def load_quad(Q, split=False):
                t8 = pool.tile([P, 4 * rows], FP8E3, tag="EQ", bufs=E_BUFS,
                               name=f"eq{Q}")
                if split:
                    # first quad lands in halves so pair 0 starts sooner
                    nc.scalar.dma_start(t8[:, : 2 * rows], eQ_d[Q, :, : 2 * rows])
                    nc.scalar.dma_start(t8[:, 2 * rows :], eQ_d[Q, :, 2 * rows :])
                else:
                    nc.scalar.dma_start(t8, eQ_d[Q])
                eQ_sb[Q] = t8

            def load_pP(q):
                t = pool.tile([P, 2, 2, rows], FP8E4, tag="PP", bufs=P_BUFS,
                              name=f"pp{q}")
                nc.sync.dma_start(t, pP_d[q])
                pPs[q] = t

            def upconvert(q):
                """One pair of E': fp8e3 -> bf16 with the +0.5 shift."""
                Q, half = q // 2, q % 2
                src = eQ_sb[Q][:, half * 2 * rows : (half + 1) * 2 * rows]
                dst = pool.tile([P, 2 * rows], BF16, tag="EB", bufs=EB_BUFS,
                                name=f"eb{q}")
                if q % 2 == 0:
                    nc.scalar.activation(
                        dst, src, mybir.ActivationFunctionType.Copy, bias=0.5
                    )
                else:
                    nc.vector.tensor_scalar_add(dst, src, 0.5)
                if half == 1:
                    eQ_sb.pop(Q)
                return dst

            # ---------------- head ----------------
            # critical-path first: xmsg[:4] leads the sync ring (first agg
            # LDWEIGHTS), E' quad 0 leads the act ring (first upconvert);
            # late-needed constants ride the otherwise-idle gpsimd SWDGE
            # ring; the rest of xmsg spreads through the stream so early
            # HWDGE bandwidth goes to pair data
            xmsg = cpool.tile([P, n_tile, f], BF16)
            nc.sync.dma_start(xmsg[:, :4], xm_d[:, :4])
            load_quad(0, split=True)
            w1d2 = cpool.tile([P, 2, F_OUT], FP8E4)
            nc.gpsimd.dma_start(w1d2, w1d2_d[:, :, :])
            load_pP(0)
            load_quad(1)
            load_pP(1)
            nc.sync.dma_start(xmsg[:, 4:8], xm_d[:, 4:8])
            w2T = cpool.tile([f, F_OUT], BF16)
            nc.gpsimd.dma_start(w2T, w2T_d[:, :])
            nc.sync.dma_start(xmsg[:, 8:16], xm_d[:, 8:16])
            xTs_sb = cpool.tile([f, rows], BF16)
            nc.gpsimd.dma_start(xTs_sb, xTs_d[:, :])

            # ---------------- main loop ----------------
            agg = pp.tile([P, rows], F32, tag="agg", bufs=1, name="agg")

            def agg_mms(q, eb):
                for t in range(2):
                    for h in range(2):
                        sl = slice(h * 512, (h + 1) * 512)
                        nc.tensor.matmul(
                            agg[:, sl],
                            xmsg[:, 2 * q + t],
                            eb[:, t * rows + h * 512 : t * rows + (h + 1) * 512],
                            start=q == 0 and t == 0,
                            stop=False,
                        )

            def dist_mms(q, h_only=None):
                pt = pPs[q]
                for t in range(2):
                    for h in (range(2) if h_only is None else [h_only]):
                        sl = slice(h * 512, (h + 1) * 512)
                        nc.tensor.matmul(
                            agg[:, sl],
                            w1d2[:, :, :],
                            pt[:, t, :, sl],
                            start=False,
                            stop=False,
                            perf_mode=mybir.MatmulPerfMode.DoubleRow,
                        )

            next_pP = 2
            next_xm = 16
            for g in range(n_grp):
                group = range(g * GROUP, (g + 1) * GROUP)
                last = g == n_grp - 1
                for q in group:
                    if q % 2 == 0 and q // 2 + PF_QUADS < n_quad:
                        load_quad(q // 2 + PF_QUADS)
                    while next_pP <= min(q + PF_PAIRS, n_pair - 1):
                        load_pP(next_pP)
                        next_pP += 1
                    # xmsg arrives in coarse chunks well ahead of its
                    # consumers (ring runs ~5 pairs ahead of the PE),
                    # freeing early bandwidth for pair data
                    if q in (2, 10):
                        hi = {2: 40, 10: n_tile}[q]
                        nc.sync.dma_start(
                            xmsg[:, next_xm:hi], xm_d[:, next_xm:hi]
                        )
                        next_xm = hi
                    eb = upconvert(q)
                    agg_mms(q, eb)
                if not last:
                    # one w1d2 stationary covers the group's dist matmuls
                    for q in group:
                        dist_mms(q)
                        pPs.pop(q)
                else:
                    # final pair splits h0/h1 so half 0's copy + store
                    # overlap the last matmuls
                    out_sb = pool.tile([P, rows], F32, tag="osb", bufs=1)
                    for q in group[:-1]:
                        dist_mms(q)
                    qf = group[-1]
                    dist_mms(qf, h_only=0)
                    nc.tensor.matmul(
                        agg[:, 0:512], w2T, xTs_sb[:, 0:512],
                        start=False, stop=True,
                    )
                    nc.scalar.copy(out_sb[:, 0:512], agg[:, 0:512])
                    nc.sync.dma_start(o_d[:, 0:512], out_sb[:, 0:512])
                    dist_mms(qf, h_only=1)
                    nc.tensor.matmul(
                        agg[:, 512:1024], w2T, xTs_sb[:, 512:1024],
                        start=False, stop=True,
                    )
                    nc.scalar.copy(out_sb[:, 512:1024], agg[:, 512:1024])
                    nc.sync.dma_start(o_d[:, 512:1024], out_sb[:, 512:1024])

    nc.compile()
    return nc


def _prep_inputs(inputs, n, rows_per_core, n_cores):
    """Host-side shard + layout + dtype prep (numpy only)."""
    bf16 = ml_dtypes.bfloat16
    # e4m3fn: bit-identical to TRN FP8_EXP4 for |x| <= 240 (all our data)
    fp8e4 = ml_dtypes.float8_e4m3fn
    fp8e3 = ml_dtypes.float8_e3m4
    x = np.asarray(inputs["x"], dtype=np.float32)
    edges = np.asarray(inputs["edges"], dtype=np.float32)
    dist = np.asarray(inputs["distance_matrix"], dtype=np.float32)
    w1 = np.asarray(inputs["w1"], dtype=np.float32)
    w2 = np.asarray(inputs["w2"], dtype=np.float32)
    f = x.shape[1]
    R = rows_per_core

    xT = np.ascontiguousarray(x.T).astype(bf16)            # [f, n]
    w2T = np.ascontiguousarray(w2.T).astype(bf16)          # [k, F_OUT]
    # DoubleRow stationary: w1d2[j, c, fout] = w1[fout, f+c] for all j
    w1d2 = np.ascontiguousarray(
        np.broadcast_to(w1[:, f : f + 2].T[None, :, :], (P, 2, w1.shape[0]))
    ).astype(fp8e4)
    # xmsg[j, fout] = x @ w1x.T, chunked [128, n/128, f] with j = chunk*128+p
    xmsg = (x @ w1[:, :f].T).astype(bf16)
    xmsg_c = np.ascontiguousarray(
        xmsg.reshape(n // P, P, f).transpose(1, 0, 2)
    )

    in_maps = []
    for c in range(n_cores):
        i0 = c * R
        i1 = i0 + R
        eT = np.ascontiguousarray(edges[i0:i1].T)          # [n, R] f32
        # E' = e - 0.5 quantizes ~2x better in e3m4; quad-packed so each
        # partition row carries 4 KB contiguous per DMA
        eQ = (eT - 0.5).astype(fp8e3)
        eQ = np.ascontiguousarray(
            eQ.reshape(n // (4 * P), 4, P, R).transpose(0, 2, 1, 3)
        ).reshape(n // (4 * P), P, 4 * R)
        # P^T[j, c, i] = e[i,j] * d[i,j,c], pair-packed [q, p, t, c, i]
        dT = np.ascontiguousarray(dist[i0:i1].transpose(1, 2, 0))  # [n, 2, R]
        pT = (eT[:, None, :] * dT).astype(fp8e4)           # [n, 2, R] fp8
        pP = np.ascontiguousarray(
            pT.reshape(n // (2 * P), 2, P, 2, R).transpose(0, 2, 1, 3, 4)
        )
        in_maps.append(
            {
                "eQ": eQ,
                "pP": pP,
                "xmsg": xmsg_c,
                "xT_self": np.ascontiguousarray(xT[:, i0:i1]),
                "w2T": w2T,
                "w1d2": w1d2,
            }
        )
    return in_maps


def _run(inputs, n, rows_per_core, n_cores, trace=False):
    from concourse.bass_utils import run_bass_kernel_spmd

    in_maps = _prep_inputs(inputs, n, rows_per_core, n_cores)
    nc = build(n=n, rows=rows_per_core)
    res = run_bass_kernel_spmd(nc, in_maps, core_ids=list(range(n_cores)), trace=trace)

    global LAST_RESULT
    LAST_RESULT = res

    out = np.concatenate([r["outT"].T for r in res.results], axis=0)
    return np.ascontiguousarray(out, dtype=np.float32)


def kernel(**inputs) -> np.ndarray:
    trace = os.environ.get("KERNEL_TRACE", "0") == "1"
    return _run(
        inputs,
        n=N_FULL,
        rows_per_core=N_FULL // N_CORES,
        n_cores=N_CORES,
        trace=trace,
    )
